# revision 81
# baseline (speedup 1.0000x reference)
"""Routed-MoE kernel for Trainium2 (8 NeuronCores).

The reference computes all-experts MLP logits for every token and then
gathers the expert chosen by `domain`.  Only the selected expert's output is
needed, so this kernel routes on the host (argsort by expert) and runs one
expert per NeuronCore over its (capacity-padded) token group:

    core e:  out = softmax(relu(Xg[e] @ W1[e] + b1[e]) @ W2[e] + b2[e])

Key layout/performance choices:
  - No on-device transposes on the main path: L1 computes H^T [F2, tok]
    with lhsT = W1 tiles (host pre-swizzled per-m so one DMA unlocks one
    m-group) and rhs = Xg^T (host-transposed gather, streamed per
    (k, token-slice) so the PE can start after ~2 MB of DMA).
  - All matmuls run in float32r (FP22 single-pass mode, 4x true-FP32
    throughput; PSUM accumulation stays FP32).  Token slices are 384 wide
    so every matmul has output free dim >= 256 (full f32r rate).
  - L2 is computed as logits^T [C, tok] (free dim = tokens >= 256), then
    PE-transposed per 128-token tile for the free-axis softmax.
  - Token slices (0,1) are interleaved inside the m-loop so PE never
    outpaces the W1 DMA stream; slice 2 runs after weights are resident.
"""

import numpy as np
from contextlib import ExitStack

import concourse.bass as bass
import concourse.bacc as bacc
import concourse.tile as tile
from concourse import mybir
from concourse.bass_utils import run_bass_kernel_spmd
from concourse.masks import make_identity

B, E, F1, F2, C = 8192, 8, 1024, 2048, 100
N_CORES = 8
P = 128
# Token-slice widths: every slice >= 256 keeps f32r matmuls at full rate.
# Slices 0/1 are small so the DMA bytes gating the PE start (x slices 0+1
# + first W1 tile) are minimal and the cold-p-state first m-group paces out
# slice 1's arrival: the PE runs one long stream with a single ramp.
SLICES = [(0, 256), (256, 256), (512, 256), (768, 256)]
CAP = 1024          # capacity factor 1.0 (binomial mean 1024, sd ~30);
                    # overflow tokens get the exact fp32 host fallback
K1 = F1 // P        # 8  K-tiles for layer 1
M1 = F2 // P        # 16 M-tiles for layer 1 (= K-tiles for layer 2)

F32 = mybir.dt.float32
F32R = mybir.dt.float32r
WARMUP_MMS = 14   # junk matmuls before the first real m-group
JUNK2 = 9         # fill between real groups (0,s0) and (0,s1)
JUNK3 = 3         # fill between (0,s1) and (1,s0)

_CACHED_NC = None


def _build_nc():
    nc = bacc.Bacc("TRN2", target_bir_lowering=False, debug=False,
                   num_devices=N_CORES)
    xT_d = nc.dram_tensor("xT", [F1, CAP], F32R, kind="ExternalInput").ap()
    # w1s host-swizzled: w1s[m*128 + p, k*128 + j] = W1[k*128 + p, m*128 + j]
    w1_d = nc.dram_tensor("w1s", [F2, F1], F32R, kind="ExternalInput").ap()
    b1_d = nc.dram_tensor("b1r", [P, M1], F32, kind="ExternalInput").ap()
    w2_d = nc.dram_tensor("w2", [F2, C], F32R, kind="ExternalInput").ap()
    b2_d = nc.dram_tensor("b2c", [C, 1], F32, kind="ExternalInput").ap()
    out_d = nc.dram_tensor("out", [CAP, C], F32, kind="ExternalOutput").ap()

    with tile.TileContext(nc) as tc, ExitStack() as ctx:
        const = ctx.enter_context(tc.tile_pool(name="const", bufs=1))
        hpool = ctx.enter_context(tc.tile_pool(name="h", bufs=1))
        ps1 = ctx.enter_context(tc.tile_pool(name="ps1", bufs=2, space="PSUM"))
        ps2 = ctx.enter_context(tc.tile_pool(name="ps2", bufs=3, space="PSUM"))
        ps3 = ctx.enter_context(tc.tile_pool(name="ps3", bufs=3, space="PSUM"))
        lpool = ctx.enter_context(tc.tile_pool(name="l2sb", bufs=4))
        spool = ctx.enter_context(tc.tile_pool(name="stats", bufs=8))
        opool = ctx.enter_context(tc.tile_pool(name="out", bufs=4))
        ppool = ctx.enter_context(tc.tile_pool(name="prob", bufs=1))

        ident = const.tile([P, P], F32, tag="ident")
        make_identity(nc, ident[:])

        # One DMA per x slice / per w1 m-group (the HWDGE issue slot is
        # ~0.6us per DMACopy, so merge everything that doesn't need
        # streaming granularity).  x source viewed as [p, k, t].
        xsrc = xT_d.rearrange("(k p) c -> p k c", k=K1)
        w2src = w2_d.rearrange("(m p) c -> p m c", m=M1)

        def load_x(s):
            n0, S = SLICES[s]
            t = const.tile([P, K1, S], F32R, tag=f"x_{s}", name=f"x_{s}")
            nc.sync.dma_start(t[:], xsrc[:, :, n0:n0 + S])
            return t

        xt = {0: load_x(0)}
        w1t = []
        b1sb = const.tile([P, M1], F32, tag="b1")
        for m in range(M1):
            t = const.tile([P, K1 * P], F32R, tag=f"w1_{m}", name=f"w1_{m}")
            nc.sync.dma_start(t[:], w1_d[m * P:(m + 1) * P, :])
            w1t.append(t)
            if m == 0:
                nc.sync.dma_start(b1sb[:], b1_d[:])
                xt[1] = load_x(1)
        xt[2] = load_x(2)
        w2sb = const.tile([P, M1, C], F32R, tag="w2")
        nc.sync.dma_start(w2sb[:], w2src[:])
        b2sb = const.tile([P, 1], F32, tag="b2")
        nc.sync.dma_start(b2sb[:C, :], b2_d[:])
        xt[3] = load_x(3)

        # H^T for the whole batch: h[p, m*CAP + n0 + t]
        h = hpool.tile([P, M1 * CAP], F32R, tag="h")

        # PE warm-up / gap filler: throwaway matmuls on a DVE-memset junk
        # tile (ready at ~0.2us, no DMA deps).  Junk blocks fill the
        # DMA-imposed waits before the first few real m-groups, so the PE is
        # continuously busy from ~0.3us on and its p-state ramp completes on
        # junk work — every real matmul then runs at full clock.
        jt = const.tile([P, P], F32, tag="junk")
        nc.vector.memset(jt[:], 1.0)
        psw = ps3.tile([P, P], F32, tag="pst", name="warm")

        def junk(n, pin_us=None, fine=0):
            # pin_us: scheduling-time floor so the Tile scheduler can't
            # hoist this filler block ahead of blocked real matmuls.
            # fine: extra quarter-length junk matmuls for fine trimming.
            def emit():
                for _ in range(n):
                    nc.tensor.matmul(psw[:], jt[:], jt[:],
                                     start=True, stop=True)
                for _ in range(fine):
                    nc.tensor.matmul(psw[:, :32], jt[:], jt[:, :32],
                                     start=True, stop=True)
            if pin_us is None:
                emit()
            else:
                with tc.tile_wait_until(pin_us * 1e-3):
                    emit()

        junk(WARMUP_MMS, fine=2)

        def l1_group(m, s):
            n0, S = SLICES[s]
            ps = ps1.tile([P, S], F32, tag="ps1", name=f"ps1_{m}_{s}")
            for k in range(K1):
                nc.tensor.matmul(
                    ps[:], w1t[m][:, k * P:(k + 1) * P], xt[s][:, k, :],
                    start=(k == 0), stop=(k == K1 - 1))
            nc.scalar.activation(
                h[:, m * CAP + n0: m * CAP + n0 + S], ps[:],
                mybir.ActivationFunctionType.Relu, bias=b1sb[:, m:m + 1])

        # L2 computes logits^T [C, sub] at full f32r rate (free dim = tokens
        # >= 256).  Its per-m accumulation matmuls are interleaved into the
        # L1 m-loop (lag 1 so the relu is done), leaving only one L2 matmul
        # group after the last L1 group -> short serial tail.
        psls = {}

        def l2_prep(s):
            n0, S = SLICES[s]
            psls[s] = [(ps2.tile([C, S], F32, tag="psl", name=f"psl_{s}"),
                        n0, S)]

        def l2_mm(s, m, sub=None):
            for i, (psl, hc0, S_sub) in enumerate(psls[s]):
                if sub is not None and i != sub:
                    continue
                nc.tensor.matmul(
                    psl[:], w2sb[:, m, :],
                    h[:, m * CAP + hc0: m * CAP + hc0 + S_sub],
                    start=(m == 0), stop=(m == M1 - 1))

        def chain(s, j0, sub=0):
            # b2 added during the DVE evict, then a PE-transpose and the
            # free-axis softmax.  No max-subtraction: logits are O(1) for
            # randn-scaled inputs (host guards isfinite and falls back).
            psl, hc0, S_sub = psls[s][sub]
            W = min(P, S_sub - j0)
            l2sb = lpool.tile([C, P], F32, tag="l2sb",
                              name=f"l2sb_{hc0}_{j0}")
            nc.vector.tensor_scalar_add(l2sb[:, :W], psl[:, j0:j0 + W],
                                        b2sb[:C, :])
            pst = ps3.tile([P, C], F32, tag="pst", name=f"pst_{hc0}_{j0}")
            nc.tensor.transpose(pst[:W, :], l2sb[:, :W], ident[:C, :C])
            ex = opool.tile([P, C], F32, tag="ex")
            sm = spool.tile([P, 1], F32, tag="sm")
            nc.scalar.activation(ex[:W, :], pst[:W, :],
                                 mybir.ActivationFunctionType.Exp,
                                 accum_out=sm[:W, :])
            rc = spool.tile([P, 1], F32, tag="rc")
            nc.vector.reciprocal(rc[:W, :], sm[:W, :])
            prob = ppool.tile([P, C], F32, tag="prob", bufs=4)
            nc.vector.tensor_scalar_mul(prob[:W, :], ex[:W, :], rc[:W, :])
            row0 = hc0 + j0
            nc.sync.dma_start(out_d[row0:row0 + W, :], prob[:W, :])

        # Phase A: slices 0/1 interleaved in the m-loop (the cold-p-state
        # first m-group paces out x slice 1's arrival).  Each phase's L2
        # accumulation matmuls run as a contiguous block right after its
        # last m-group (h complete, PE stream contiguous); each slice's
        # softmax chains are threaded between the NEXT phase's m-groups so
        # their DVE evicts get a full m-group of slack and the PE
        # transposes slot in gap-free.  The final slice is small so its
        # trailing chains are short.
        l2_prep(0)
        l2_prep(1)
        l1_group(0, 0)
        junk(JUNK2, pin_us=8.1, fine=1)
        l1_group(0, 1)
        junk(JUNK3, pin_us=11.0)
        for m in range(1, M1):
            l1_group(m, 0)
            l1_group(m, 1)
        for m in range(M1):
            l2_mm(0, m)
        for m in range(M1):
            l2_mm(1, m)

        l2_prep(2)
        ab_chunks = [(s, j0) for s in (0, 1)
                     for j0 in range(0, SLICES[s][1], P)]
        l1_group(0, 2)
        for m in range(1, M1):
            l1_group(m, 2)
            if 0 <= m - 2 < len(ab_chunks):
                chain(*ab_chunks[m - 2])
        for m in range(M1):
            l2_mm(2, m)

        l2_prep(3)
        c_chunks = [(2, j0) for j0 in range(0, SLICES[2][1], P)]
        l1_group(0, 3)
        for m in range(1, M1):
            l1_group(m, 3)
            if 0 <= m - 2 < len(c_chunks):
                chain(*c_chunks[m - 2])
        for m in range(M1):
            l2_mm(3, m)
        chain(3, 0)
        chain(3, P)

    nc.compile()
    return nc


def _get_nc():
    global _CACHED_NC
    if _CACHED_NC is None:
        _CACHED_NC = _build_nc()
    return _CACHED_NC


def _np_mlp_rows(x_rows, e, W1, b1, W2, b2):
    """Host fallback (exact fp32 semantics) for capacity-overflow tokens."""
    h = np.maximum(x_rows.astype(np.float32) @ W1[e] + b1[e], 0.0)
    logits = h @ W2[e] + b2[e]
    logits -= logits.max(axis=-1, keepdims=True)
    p = np.exp(logits)
    return (p / p.sum(axis=-1, keepdims=True)).astype(np.float32)


def kernel(domain, x, W1, b1, W2, b2):
    # Clamp like jnp.take_along_axis does for out-of-range indices.
    domain = np.clip(np.asarray(domain).astype(np.int64), 0, E - 1)
    x = np.ascontiguousarray(np.asarray(x, dtype=np.float32))
    W1 = np.asarray(W1, dtype=np.float32)
    b1 = np.asarray(b1, dtype=np.float32)
    W2 = np.asarray(W2, dtype=np.float32)
    b2 = np.asarray(b2, dtype=np.float32)

    order = np.argsort(domain, kind="stable")
    counts = np.bincount(domain, minlength=E).astype(np.int64)
    starts = np.concatenate([[0], np.cumsum(counts)[:-1]])

    xT = x.T  # [F1, B] view
    in_maps = []
    kept_idx = []
    for e in range(E):
        n_e = int(min(counts[e], CAP))
        idx = order[starts[e]: starts[e] + n_e]
        kept_idx.append(idx)
        xg = np.zeros((F1, CAP), dtype=np.float32)
        xg[:, :n_e] = xT[:, idx]
        # w1s[m*128+p, k*128+j] = W1[e][k*128+p, m*128+j]
        w1s = np.ascontiguousarray(
            W1[e].reshape(K1, P, M1, P).transpose(2, 1, 0, 3).reshape(F2, F1))
        in_maps.append({
            "xT": xg,
            "w1s": w1s,
            "b1r": np.ascontiguousarray(b1[e].reshape(M1, P).T),
            "w2": np.ascontiguousarray(W2[e]),
            "b2c": np.ascontiguousarray(b2[e].reshape(C, 1)),
        })

    nc = _get_nc()
    results = run_bass_kernel_spmd(nc, in_maps, list(range(N_CORES))).results

    out = np.empty((B, C), dtype=np.float32)
    for e in range(E):
        idx = kept_idx[e]
        out[idx] = results[e]["out"][: len(idx)]
        if counts[e] > CAP:  # astronomically unlikely; exact host fallback
            ov = order[starts[e] + CAP: starts[e] + counts[e]]
            out[ov] = _np_mlp_rows(x[ov], e, W1, b1, W2, b2)

    # The device softmax skips max-centering (logits are O(1) for
    # randn-scaled inputs); guard against overflow just in case.
    bad = ~np.isfinite(out).all(axis=1)
    if bad.any():
        for i in np.nonzero(bad)[0]:
            out[i] = _np_mlp_rows(x[i:i + 1], int(domain[i]), W1, b1, W2, b2)

    return out
